# revision 78
# baseline (speedup 1.0000x reference)
"""Trainium2 Bass kernel for nn_Bottleneck_57561151701110 (SAM pairwise
bottleneck block). Data-parallel over batch: 8 images -> 8 NeuronCores.

v3 (~105us vs 121us v2 baseline):
 - fp16 end-to-end: x in / out in fp16 (HBM traffic halved); the +x
   residual is added on the host in f32 (also improves accuracy).
 - conv1/conv2 computed directly in band layout via PE column-tiling
   (tile_position col groups), killing the x12s band-remap DMAs; the
   x2bB shifted copy becomes a second eviction of the same PSUM.
 - xgB eliminated: dw=0 aggregation planes multiply the full 58-wide
   xgA rows against a column-shifted copy of e (esh, one extra ACT
   write per dw=0 k); the identity matmul reads the odd-offset window
   (PE has no alignment constraint), so every DVE multiply stays in
   2x mode with no shifted xg copy and no DMA chain.
 - PE warmup runs on a DVE-memset tile with no DMA dependency, so HAM
   unthrottles to 2.4 GHz before conv1 (K=8/8 from ~10us).
 - evictions balanced: conv3 t=0 ACT / t=1 DVE; hp fully DVE; e/esh
   on ACT (its exp chain paces phase C).
 - sam remaps issue per s-quad (samsb channel order permuted so each
   quad's 64 dst partitions are contiguous; convo lhsT rows permuted
   to match), and the last quad's issues split across both HWDGE
   queues -- the tail remap latency roughly halves.

Per-core pipeline (one 256x56x56 image):
  conv1/conv2 (PE fp16, 4-band col-tiles) -> x1b/x2b/x2bB (ACT evict) ->
  conv3 (PE) -> x3sf fp16 (58x58 zero-border planes) -> xgA remaps
  per k: fr = relu(x1b - x2win) (DVE) -> w1/pos/w2 blockdiag matmuls
    (PE) -> hp relu (DVE) / e exp + esh (ACT) -> Z (PE identity)
  rz = exp(-ln(Z)) (ACT) -> agg: pk = xg*e (DVE), k-sum (PE identity),
  Prelu (ACT), *rz (DVE) -> sqf -> sam remap (DMA) -> convo (PE) ->
  Prelu+bias (ACT) -> out fp16 -> host adds +x in f32.
"""

import os
import sys

for _p in ("/opt/trn_rl_repo", os.path.expanduser("~/.axon_site/_ro/trn_rl_repo")):
    if os.path.isdir(_p) and _p not in sys.path:
        sys.path.insert(0, _p)

from contextlib import ExitStack

import numpy as np

import concourse.bass as bass
import concourse.bacc as bacc
import concourse.tile as tile
from concourse import mybir
from concourse.bass_utils import run_bass_kernel_spmd

dt = mybir.dt
ALU = mybir.AluOpType
ACTF = mybir.ActivationFunctionType

B, CIN, H, W = 8, 256, 56, 56
NPIX = H * W            # 3136
REL, MID, OUT = 32, 256, 256
SHARE = 8
NB = 4                  # row bands
BH = H // NB            # 14 rows per band
Q = BH * W              # 784 band pixels
NEG = 0.01
BN_EPS = 1e-5
CCH = 448               # conv free chunk (8 rows)
NCH = NPIX // CCH       # 7
OFFS = [(dh, dw) for dh in (-1, 0, 1) for dw in (-1, 0, 1)]
# phase-C/E processing order: dh=0 first (no edge-row dependency in x2b),
# dw=0 offsets last (they need x2bB / the shifted-e copies)
OFFS_ORD = [3, 5, 0, 2, 6, 8, 1, 4, 7]
NSQ = SHARE // 2        # 4 s per quad
HBW = Q // 2            # 392 half-band pixels (7 rows)

F32, BF16 = dt.float32, dt.float16  # 16-bit = fp16 (values < 16)

_CACHE = {}

# packed const layouts (cols)
#  cb (fp16, 128 part): c1T (2*32) | c2T (2*32) | c3 (2*256) | co (2*256)
#                        | w1 (128) | w2 (128) | ident (128)
CB_C1, CB_C2, CB_C3, CB_CO = 0, 64, 128, 640
CB_W1, CB_W2, CB_ID = 1152, 1280, 1408
CB_COLS = 1536
#  cf (f32, 128 part): 8 bias vecs
#   0: b1 (tiled)  1: b2 (tiled)  2: conv3_b[:128]  3: conv3_b[128:]
#   4: betab (tiled)  5: w2_b (tiled)  6: biaso[:128]  7: biaso[128:]
CF_COLS = 8
#  pb (fp16, 8 part): post (128) | posr (9*784)
PB_POST, PB_POSR = 0, 128
PB_COLS = 128 + 9 * Q

# band b needs x rows 14b-1..14b+15 -> conv chunks 0..band_last_chunk[b]
BAND_LAST_CHUNK = [1, 3, 5, 6]
# conv3 runs in 4 chunks of 14 rows; band b needs chunks 0..BAND3_LAST[b]
BAND3_LAST = [1, 2, 3, 3]


# ----------------------------------------------------------------- host prep
def _position(h, w):
    loc_w = np.tile(np.linspace(-1.0, 1.0, w, dtype=np.float32)[None, :], (h, 1))
    loc_h = np.tile(np.linspace(-1.0, 1.0, h, dtype=np.float32)[:, None], (1, w))
    return np.stack([loc_w, loc_h], axis=0)  # (2, H, W)


def _host_consts(inp):
    f32 = np.float32
    f16 = np.float16
    inv_a = (inp["bna_g"] / np.sqrt(inp["bna_v"] + BN_EPS)).astype(f32)
    beta_a = (inp["bna_b"] - inp["bna_m"] * inv_a).astype(f32)
    inv_b = (inp["bnb_g"] / np.sqrt(inp["bnb_v"] + BN_EPS)).astype(f32)
    beta_b = (inp["bnb_b"] - inp["bnb_m"] * inv_b).astype(f32)

    w1c = inp["conv1_w"] * inv_a[:REL, None]          # (32, 256)
    b1 = inp["conv1_b"] * inv_a[:REL] + beta_a[:REL]
    w2c = inp["conv2_w"] * inv_a[:REL, None]
    b2 = inp["conv2_b"] * inv_a[:REL]

    # lhsT chunks: [p, t, o] = w[o, 128t+p]
    c1T = np.transpose(w1c.T.reshape(2, 128, REL), (1, 0, 2))   # (128, 2, 32)
    c2T = np.transpose(w2c.T.reshape(2, 128, REL), (1, 0, 2))
    c3 = np.transpose(inp["conv3_w"].T.reshape(2, 128, MID), (1, 0, 2))
    # convo lhsT rows permuted to match the per-s-quad sam remap layout:
    # samsb partition p' = 64*(s//4) + 4*g + (s%4) holds channel
    # c = 128t + 8g + s  (g = group-within-half, s = share index)
    perm = np.array([8 * ((p % 64) // 4) + 4 * (p // 64) + (p % 4)
                     for p in range(128)])
    cw = inp["convo_w"]
    co = np.zeros((128, 2, OUT), np.float32)
    for tk in range(2):
        co[:, tk, :] = cw[:, 128 * tk + perm].T

    # W1' with bnb scale folded
    w1p = (inp["w1"] * inv_b[:, None]).astype(f32)  # (32, 34)
    w1a, w1b = w1p[:, :REL], w1p[:, REL:]
    lhsT_w1 = np.zeros((128, 128), f32)
    lhsT_pos = np.zeros((8, 128), f32)
    lhsT_w2 = np.zeros((128, 128), f32)
    for b in range(NB):
        lhsT_w1[32 * b:32 * b + 32, 32 * b:32 * b + 32] = w1a.T  # [c, o]
        lhsT_pos[2 * b:2 * b + 2, 32 * b:32 * b + 32] = w1b.T    # [c2, o]
        lhsT_w2[32 * b:32 * b + 32, 32 * b:32 * b + 32] = inp["w2"].T  # [o, g]

    # position branch, batch independent: posr[2b+c2, 784k+q] =
    # relu(inv_a[32+c2]*subp[c2,k,band b pix q] + beta_a[32+c2])
    pos = _position(H, W)
    pc = np.einsum("oc,chw->ohw", inp["convp_w"], pos) + inp["convp_b"][:, None, None]
    pcp = np.pad(pc, ((0, 0), (1, 1), (1, 1)))
    posr = np.zeros((8, 9 * Q), f32)
    for k, (dh, dw) in enumerate(OFFS):
        sub = pc - pcp[:, 1 + dh:1 + dh + H, 1 + dw:1 + dw + W]  # (2,56,56)
        v = np.maximum(inv_a[REL:, None, None] * sub + beta_a[REL:, None, None], 0.0)
        vb = v.reshape(2, NB, BH, W)  # (c2, b, r, w)
        for b in range(NB):
            posr[2 * b:2 * b + 2, Q * k:Q * (k + 1)] = vb[:, b].reshape(2, Q)

    vecs = np.zeros((128, CF_COLS), f32)
    vecs[:, 0] = np.tile(b1, NB)
    vecs[:, 1] = np.tile(b2, NB)
    vecs[:, 2] = inp["conv3_b"][:128]
    vecs[:, 3] = inp["conv3_b"][128:]
    vecs[:, 4] = np.tile(beta_b, NB)
    vecs[:, 5] = np.tile(inp["w2_b"], NB)
    vecs[:, 6] = inp["convo_b"][:128]
    vecs[:, 7] = inp["convo_b"][128:]

    cb = np.zeros((128, CB_COLS), f16)
    cb[:, CB_C1:CB_C1 + 64] = c1T.reshape(128, 64)
    cb[:, CB_C2:CB_C2 + 64] = c2T.reshape(128, 64)
    cb[:, CB_C3:CB_C3 + 512] = c3.reshape(128, 512)
    cb[:, CB_CO:CB_CO + 512] = co.reshape(128, 512)
    cb[:, CB_W1:CB_W1 + 128] = lhsT_w1.astype(f16)
    cb[:, CB_W2:CB_W2 + 128] = lhsT_w2.astype(f16)
    cb[:, CB_ID:CB_ID + 128] = np.eye(128, dtype=f16)

    pb = np.zeros((8, PB_COLS), f16)
    pb[:, PB_POST:PB_POST + 128] = lhsT_pos.astype(f16)
    pb[:, PB_POSR:] = posr.astype(f16)

    return {"cf": vecs, "cb": cb, "pb": pb}


DEBUG = os.environ.get("KDEBUG", "0") == "1"


# ------------------------------------------------------------ program build
def _build_program():
    nc = bacc.Bacc("TRN2", target_bir_lowering=False, debug=False,
                   enable_asserts=False, num_devices=8)

    xind = nc.dram_tensor("xin", [CIN, NPIX], BF16, kind="ExternalInput").ap()
    cfd = nc.dram_tensor("cf", [128, CF_COLS], F32, kind="ExternalInput").ap()
    cbd = nc.dram_tensor("cb", [128, CB_COLS], BF16, kind="ExternalInput").ap()
    pbd = nc.dram_tensor("pb", [8, PB_COLS], BF16, kind="ExternalInput").ap()
    outd = nc.dram_tensor("out", [CIN, NPIX], BF16, kind="ExternalOutput").ap()
    dbg = {}
    if DEBUG:
        for nm, shape in [("d_x1b", [128, Q]), ("d_x2b", [128, 16 * 58]),
                          ("d_x2bB", [128, 16 * 58]),
                          ("d_x3sf", [128, 2 * 58 * 58]),
                          ("d_e0", [128, Q]), ("d_e4", [128, Q]),
                          ("d_rz", [128, Q]), ("d_sqf", [128, SHARE * Q]),
                          ("d_samsb", [128, 2 * NPIX]),
                          ("d_xga", [128, SHARE * 16 * 58]),
                          ("d_xgb", [128, SHARE * 16 * 58])]:
            dbg[nm] = nc.dram_tensor(nm, shape, BF16, kind="ExternalOutput").ap()

    with tile.TileContext(nc) as tc, ExitStack() as ctx:
        nc_ = tc.nc

        # ---- persistent pools
        cpool = ctx.enter_context(tc.tile_pool(name="consts", bufs=1))
        xpool = ctx.enter_context(tc.tile_pool(name="xin", bufs=1))
        sbp = ctx.enter_context(tc.tile_pool(name="sbp", bufs=1))
        epool = ctx.enter_context(tc.tile_pool(name="epool", bufs=9))

        # ---- const loads: 3 packed DMAs on the scalar queue
        cbt = cpool.tile([128, CB_COLS], BF16, tag="cb")
        nc.scalar.dma_start(cbt[:], cbd[:])
        cft = cpool.tile([128, CF_COLS], F32, tag="cf")
        nc.scalar.dma_start(cft[:], cfd[:])
        pbt = cpool.tile([8, PB_COLS], BF16, tag="pb")
        nc.scalar.dma_start(pbt[:], pbd[:])

        c1t = cbt[:, CB_C1:CB_C1 + 64].rearrange("p (t o) -> p t o", t=2)
        c2t = cbt[:, CB_C2:CB_C2 + 64].rearrange("p (t o) -> p t o", t=2)
        c3t = cbt[:, CB_C3:CB_C3 + 512].rearrange("p (t o) -> p t o", t=2)
        cot = cbt[:, CB_CO:CB_CO + 512].rearrange("p (t o) -> p t o", t=2)
        w1t = cbt[:, CB_W1:CB_W1 + 128]
        w2t = cbt[:, CB_W2:CB_W2 + 128]
        identt = cbt[:, CB_ID:CB_ID + 128]
        vecst = cft
        post = pbt[:, PB_POST:PB_POST + 128]
        prt = pbt[:, PB_POSR:].rearrange("p (k q) -> p k q", k=9)

        # ---- x input: one resident fp16 tile, 7 chunk DMAs (sync queue)
        xfull = xpool.tile([128, 2, NPIX], BF16, tag="xfull")
        xin_r = xind[:].rearrange("(t p) q -> p t q", t=2)
        for c in range(NCH):
            sl = slice(CCH * c, CCH * (c + 1))
            nc.sync.dma_start(xfull[:, :, sl], xin_r[:, :, sl])

        # ---- band-layout tiles + zero borders (memsets on idle GpSimd)
        x1b = sbp.tile([128, Q], BF16, tag="x1b")
        x2b = sbp.tile([128, 16, 58], BF16, tag="x2b")
        x2bB = sbp.tile([128, 16, 58], BF16, tag="x2bB")
        x3sf = sbp.tile([128, 2, 58, 58], BF16, tag="x3sf")
        xgA = sbp.tile([128, SHARE, 16, 58], BF16, tag="xgA")
        nc_.gpsimd.memset(x2b[:], 0.0)
        nc_.gpsimd.memset(x2bB[:], 0.0)
        for t in range(2):
            nc_.gpsimd.memset(x3sf[:, t, 0:1, :], 0.0)      # top border row
            nc_.gpsimd.memset(x3sf[:, t, 57:58, :], 0.0)    # bottom border row
            nc_.gpsimd.memset(x3sf[:, t, 1:57, 0:1], 0.0)   # left border col
            nc_.gpsimd.memset(x3sf[:, t, 1:57, 57:58], 0.0)  # right border col

        # ---- PE warmup on UNINITIALIZED SBUF: zero dependencies, so the
        # matmuls start the moment the PE engine boots (garbage/NaN results
        # land in a dummy PSUM bank that is never read). ~6.4us of activity
        # bridges the input-DMA wait and unthrottles HAM to 2.4 GHz before
        # conv1 starts.
        wpool = ctx.enter_context(tc.tile_pool(name="wpool", bufs=1))
        wsrc = wpool.tile([128, 128], BF16, tag="wsrc")
        nc_.vector.memset(wsrc[:], 0.0)   # DVE is idle at boot
        wexp = wpool.tile([1, 8], F32, tag="wexp")
        nc_.vector.memset(wexp[:], 0.0)
        wdst = wpool.tile([1, 8], F32, tag="wdst")
        nc_.scalar.activation(wdst[:], wexp[:], ACTF.Exp)  # exp table warm
        with tc.tile_pool(name="pwarm", bufs=1, space="PSUM") as pwarm:
            wps_ = pwarm.tile([128, 128], F32, tag="wps")
            for _ in range(60):
                nc_.tensor.matmul(wps_[:], wsrc[:], wsrc[:],
                                  start=True, stop=True)

        # ---- phase A: conv1+conv2 directly in band layout via col-tiling.
        # x1p[32b+j, q]  = conv1 out channel j at band-b pixel q
        # x2p[32b+j, r*56+w] = conv2 out channel j at img (14b-1+r, w)
        pscope0 = ExitStack()
        pp12 = pscope0.enter_context(tc.tile_pool(name="pp12", bufs=1, space="PSUM"))
        x1p = pp12.tile([128, Q], F32, tag="x1p")
        x2p = pp12.tile([128, 16 * W], F32, tag="x2p")
        for b in range(NB):
            bsl = slice(32 * b, 32 * b + 32)
            tp = (0, 32 * b)
            # conv1: band pixels 784b..784(b+1), bank-safe col splits
            for (c0, cw) in ((0, 512), (512, Q - 512)):
                for t in range(2):
                    nc_.tensor.matmul(
                        x1p[bsl, c0:c0 + cw], c1t[:, t, :],
                        xfull[:, t, Q * b + c0:Q * b + c0 + cw],
                        start=(t == 0), stop=(t == 1), tile_position=tp)
            # conv2: rows 14b-1..14b+15 (halo), invalid edge rows skipped
            p0 = (BH * b - 1) * W            # img pixel of dst col 0
            dlo = W if b == 0 else 0         # first valid dst col
            dhi = 16 * W - W if b == NB - 1 else 16 * W
            splits = [(d0, d1) for (d0, d1) in ((dlo, 512), (512, dhi))
                      if d1 > d0]
            for (d0, d1) in splits:
                for t in range(2):
                    nc_.tensor.matmul(
                        x2p[bsl, d0:d1], c2t[:, t, :],
                        xfull[:, t, p0 + d0:p0 + d1],
                        start=(t == 0), stop=(t == 1), tile_position=tp)
        # evictions (ACT, fp16, bias folded): x1b flat; x2b/x2bB as 16x58
        # zero-border planes, data at col offset 1 (x2b) / 2 (x2bB)
        nc_.scalar.activation(x1b[:], x1p[:], ACTF.Identity, bias=vecst[:, 0:1])

        def evict_x2(xt, co_, eng):
            if eng is nc_.scalar:
                eng.activation(xt[:, 1:15, co_:co_ + W],
                               x2p[:, W:15 * W].rearrange(
                                   "p (r w) -> p r w", w=W),
                               ACTF.Identity, bias=vecst[:, 1:2])
            else:
                eng.tensor_scalar(xt[:, 1:15, co_:co_ + W],
                                  x2p[:, W:15 * W].rearrange(
                                      "p (r w) -> p r w", w=W),
                                  vecst[:, 1:2], None, op0=ALU.add)
            # edge rows. row 15 (valid for bands 0..2) is one op at base
            # partition 0; row 0 (bands 1..3) must be split per band (PSUM
            # APs at partition offset p>0 may not span more than 32
            # partitions)
            edges = [(slice(0, 96), 15, slice(15 * W, 16 * W))] + \
                    [(slice(32 * b, 32 * b + 32), 0, slice(0, W))
                     for b in range(1, NB)]
            for (bsl, r, ssl) in edges:
                if eng is nc_.scalar:
                    eng.activation(xt[bsl, r:r + 1, co_:co_ + W],
                                   x2p[bsl, ssl].unsqueeze(1),
                                   ACTF.Identity, bias=vecst[bsl, 1:2])
                else:
                    eng.tensor_scalar(xt[bsl, r:r + 1, co_:co_ + W],
                                      x2p[bsl, ssl].unsqueeze(1),
                                      vecst[bsl, 1:2], None, op0=ALU.add)

        evict_x2(x2b, 1, nc_.scalar)
        evict_x2(x2bB, 2, nc_.scalar)
        pscope0.close()

        # ---- phase B: conv3 (evictions split ACT/DVE; xgA remaps per band)
        pscope1 = ExitStack()
        ppc = pscope1.enter_context(tc.tile_pool(name="ppc", bufs=2, space="PSUM"))
        for c in range(NCH):
            sl = slice(CCH * c, CCH * (c + 1))
            for t in range(2):
                ps3 = ppc.tile([128, CCH], F32, tag="conv")
                nc_.tensor.matmul(ps3[:], c3t[:, 0, 128 * t:128 * (t + 1)],
                                  xfull[:, 0, sl], start=True, stop=False)
                nc_.tensor.matmul(ps3[:], c3t[:, 1, 128 * t:128 * (t + 1)],
                                  xfull[:, 1, sl], start=False, stop=True)
                dst3 = x3sf[:, t, 1 + 8 * c:1 + 8 * c + 8, 1:57]
                src3 = ps3[:].rearrange("p (r w) -> p r w", w=W)
                if t == 0:
                    nc_.scalar.activation(dst3, src3, ACTF.Identity,
                                          bias=vecst[:, 2:3])
                else:
                    nc_.vector.tensor_scalar(dst3, src3, vecst[:, 3:4], None,
                                             op0=ALU.add)
            # xg band remaps: band b needs x3sf plane rows 14b..14b+16
            # xgA[32b+16t+g, s, r, w] = x3sf[8g+s, t, 14b+r, w]
            # (contiguous 928-elem runs per partition -> cheap descriptors)
            for b in range(NB):
                if BAND_LAST_CHUNK[b] == c:
                    for t in range(2):
                        psl = slice(32 * b + 16 * t, 32 * b + 16 * t + 16)
                        nc.sync.dma_start(
                            xgA[psl],
                            x3sf[:, t, 14 * b:14 * b + 16, :])
        pscope1.close()

        xgo = {-1: 0, 1: 2}   # xgA col offset of the dh-window, dw != 0
        x2 = {-1: x2b, 0: x2bB, 1: x2b}
        x2o = {-1: 0, 0: 2, 1: 2}

        # ---- phase C: per-k logits + exp + Z (PE fp16 + ACT)
        frpool = ctx.enter_context(tc.tile_pool(name="frpool", bufs=4))
        hpool = ctx.enter_context(tc.tile_pool(name="hpool", bufs=3))
        eshpool = ctx.enter_context(tc.tile_pool(name="eshpool", bufs=3))
        esh_d = {}
        pscope2 = ExitStack()
        ppz = pscope2.enter_context(tc.tile_pool(name="ppz", bufs=1, space="PSUM"))
        pscope3 = ExitStack()
        pph = pscope3.enter_context(tc.tile_pool(name="pph", bufs=3, space="PSUM"))
        ek = {}
        zps = ppz.tile([128, 1024], F32, tag="zps")
        wsl = [slice(0, 512), slice(512, Q)]
        # software-pipelined: iteration i emits stage-1 work for k_i and
        # stage-2 work for k_{i-1}, so no engine FIFO head-of-line blocks.
        hp_d = {}
        for i in range(10):
            if i < 9:
                k = OFFS_ORD[i]
                dh, dw = OFFS[k]
                co_ = x2o[dw]
                fs = frpool.tile([128, BH, W], BF16, tag="fs")
                nc_.vector.tensor_tensor(
                    fs[:], x1b[:].rearrange("p (r w) -> p r w", w=W),
                    x2[dw][:, 1 + dh:1 + dh + BH, co_:co_ + W],
                    ALU.subtract)
                fr = frpool.tile([128, Q], BF16, tag="fr")
                nc_.vector.tensor_scalar(fr[:].rearrange("p (r w) -> p r w", w=W),
                                         fs[:], 0.0, None, op0=ALU.max)
                hps = pph.tile([128, 1024], F32, tag="hw")
                # same-stationary matmuls back-to-back (one weight load/pair)
                for s in wsl:
                    nc_.tensor.matmul(hps[:, s], w1t[:], fr[:, s],
                                      start=True, stop=False)
                for s in wsl:
                    nc_.tensor.matmul(hps[:, s], post[:], prt[0:8, k, s],
                                      start=False, stop=True)
                hp = hpool.tile([128, Q], BF16, tag="hp")
                hp_d[k] = hp
                # hp eviction fully on DVE: ACT is the phase-C critical
                # chain (it owns all the exp evictions)
                nc_.vector.tensor_scalar(hp[:], hps[:, 0:Q],
                                         vecst[:, 4:5], 0.0,
                                         op0=ALU.add, op1=ALU.max)
            if i >= 1:
                kp = OFFS_ORD[i - 1]
                wps = pph.tile([128, 1024], F32, tag="hw")
                for s in wsl:
                    nc_.tensor.matmul(wps[:, s], w2t[:], hp_d[kp][:, s],
                                      start=True, stop=True)
                e = epool.tile([128, Q], BF16, tag="e")
                nc_.scalar.activation(e[:], wps[:, 0:Q], ACTF.Exp,
                                      bias=vecst[:, 5:6])
                if OFFS[kp][1] == 0:
                    # dw=0 planes also need e at odd column parity: a second
                    # ACT write into a 58-wide row layout at col offset 1
                    # (esh[g, r, 1+w] = e[g, r, w]; cols 0/57 stay garbage,
                    # never read)
                    esh = eshpool.tile([128, BH, 58], BF16, tag="esh")
                    esh_d[kp] = esh
                    nc_.scalar.activation(
                        esh[:, :, 1:57],
                        wps[:, 0:Q].rearrange("p (r w) -> p r w", w=W),
                        ACTF.Exp, bias=vecst[:, 5:6])
                for s in wsl:
                    nc_.tensor.matmul(zps[:, s], identt[:], e[:, s],
                                      start=(i == 1), stop=(i == 9))
                ek[kp] = e
        pscope3.close()

        # ---- phase E: aggregation + convo, per half-band row group qp.
        # The rz = exp(-ln(Z)) computation (and its ACT table swaps) is
        # emitted AFTER the first quad's multiply/matmul rounds so the two
        # table loads hide under quad-0 compute.
        rzpool = ctx.enter_context(tc.tile_pool(name="rzpool", bufs=1))
        lnz = rzpool.tile([128, Q], F32, tag="lnz")
        rz16 = rzpool.tile([128, Q], BF16, tag="rz16")
        pkpool = ctx.enter_context(tc.tile_pool(name="pkpool", bufs=8))
        sqf = sbp.tile([128, SHARE, Q], BF16, tag="sqf")
        lkpool = ctx.enter_context(tc.tile_pool(name="lkpool", bufs=3))
        opool = ctx.enter_context(tc.tile_pool(name="opool", bufs=4))
        samsb = sbp.tile([128, 2, NPIX], BF16, tag="samsb")
        pscope4 = ExitStack()
        pps = pscope4.enter_context(tc.tile_pool(name="pps", bufs=1, space="PSUM"))
        ppo = pscope4.enter_context(tc.tile_pool(name="ppo", bufs=2, space="PSUM"))

        first_quad = [True]

        def emit_pk(k, qp, sq, engine):
            # pk plane for offset k, quad (qp, sq); returns matmul rhs list
            dh, dw = OFFS[k]
            r0 = 1 + dh + 7 * qp
            sqs = slice(NSQ * sq, NSQ * (sq + 1))
            if dw != 0:
                pk = pkpool.tile([128, NSQ, 7, W], BF16, tag="pk")
                co_ = xgo[dw]
                engine.tensor_tensor(
                    pk[:],
                    xgA[:, sqs, r0:r0 + 7, co_:co_ + W],
                    ek[k][:].rearrange("p (r w) -> p r w", w=W)
                    [:, 7 * qp:7 * qp + 7, :].unsqueeze(1)
                    .broadcast_to((128, NSQ, 7, W)),
                    ALU.mult)
                return [pk[:, c4, :, :] for c4 in range(4)]
            # dw=0: full 58-wide product against the shifted-e copy (both
            # operands at even parity); the identity matmul reads the
            # odd-offset window instead
            pk = pkpool.tile([128, NSQ, 7, 58], BF16, tag="pk8")
            engine.tensor_tensor(
                pk[:],
                xgA[:, sqs, r0:r0 + 7, 0:58],
                esh_d[k][:, 7 * qp:7 * qp + 7, :].unsqueeze(1)
                .broadcast_to((128, NSQ, 7, 58)),
                ALU.mult)
            return [pk[:, c4, :, 1:57] for c4 in range(4)]

        for qp in range(2):
            for sq in range(2):
                sam = pps.tile([128, 2048], F32, tag="sam")
                samv = sam[:].rearrange("p (a j) -> p a j", j=512)[:, :, 0:HBW]
                for j, k in enumerate(OFFS_ORD):
                    rhs = emit_pk(k, qp, sq, nc_.vector)
                    for c4 in range(4):
                        nc_.tensor.matmul(
                            sam[:, 512 * c4:512 * c4 + HBW], identt[:],
                            rhs[c4],
                            start=(j == 0), stop=(j == 8))
                if first_quad[0]:
                    # rz = exp(-ln(Z)): table loads overlap quad-0 compute
                    nc_.scalar.activation(lnz[:], zps[:, 0:Q], ACTF.Ln)
                    nc_.scalar.activation(rz16[:], lnz[:], ACTF.Exp,
                                          scale=-1.0)
                    first_quad[0] = False
                # leaky(sam) then * rz -> sqf quad slice
                lk = lkpool.tile([128, NSQ, HBW], BF16, tag="lk")
                nc_.scalar.activation(lk[:], samv, ACTF.Prelu, alpha=NEG)
                nc_.vector.tensor_tensor(
                    sqf[:, NSQ * sq:NSQ * (sq + 1), HBW * qp:HBW * (qp + 1)],
                    lk[:],
                    rz16[:, HBW * qp:HBW * (qp + 1)].unsqueeze(1)
                    .broadcast_to((128, NSQ, HBW)),
                    ALU.mult)
                # sam remap for this s-quad: dst partitions 64sq..64sq+64
                # are contiguous (convo lhsT rows permuted to match), so the
                # transfer starts one quad earlier than a full-s remap.
                # dst [64, 392] pairs with src [16, 4, 392] in flat order:
                # dst 64sq+4g+smod <- sqf[32b+16t+g, 4sq+smod]
                for t in range(2):
                    for b in range(NB):
                        # the last quad's issues split across both HWDGE
                        # queues: ACT has no independent work left there,
                        # so head-of-line blocking is harmless and the
                        # 8x ~0.7us issue serialization halves
                        q_ = (nc.scalar if qp == 1 and sq == 1
                              and (t + b) % 2 == 1 else nc.sync)
                        q_.dma_start(
                            samsb[64 * sq:64 * sq + 64, t,
                                  Q * b + HBW * qp:Q * b + HBW * (qp + 1)],
                            sqf[32 * b + 16 * t:32 * b + 16 * t + 16,
                                NSQ * sq:NSQ * (sq + 1),
                                HBW * qp:HBW * (qp + 1)])
            # convo on this qp's half-bands; one merged store per band
            # (residual +x is added on the host in f32)
            for b in range(NB):
                po = Q * b + HBW * qp
                o = opool.tile([128, 2, HBW], BF16, tag="o")
                for to in range(2):
                    pso = ppo.tile([128, HBW], F32, tag="pso")
                    nc_.tensor.matmul(pso[:], cot[:, 0, 128 * to:128 * (to + 1)],
                                      samsb[:, 0, po:po + HBW],
                                      start=True, stop=False)
                    nc_.tensor.matmul(pso[:], cot[:, 1, 128 * to:128 * (to + 1)],
                                      samsb[:, 1, po:po + HBW],
                                      start=False, stop=True)
                    nc_.scalar.activation(o[:, to, :], pso[:], ACTF.Prelu,
                                          bias=vecst[:, 6 + to:7 + to],
                                          alpha=NEG)
                nc.sync.dma_start(
                    outd[:].rearrange("(t p) q -> p t q", t=2)
                    [:, :, po:po + HBW], o[:])
        pscope4.close()
        pscope2.close()

        if DEBUG:
            nc.sync.dma_start(dbg["d_x1b"][:], x1b[:])
            nc.sync.dma_start(dbg["d_x2b"][:],
                              x2b[:].rearrange("p r w -> p (r w)"))
            nc.sync.dma_start(dbg["d_x2bB"][:],
                              x2bB[:].rearrange("p r w -> p (r w)"))
            nc.sync.dma_start(dbg["d_x3sf"][:],
                              x3sf[:].rearrange("p t r w -> p (t r w)"))
            nc.sync.dma_start(dbg["d_e0"][:], ek[0][:])
            nc.sync.dma_start(dbg["d_e4"][:], ek[4][:])
            nc.sync.dma_start(dbg["d_rz"][:], rz16[:])
            nc.sync.dma_start(dbg["d_sqf"][:],
                              sqf[:].rearrange("p s q -> p (s q)"))
            nc.sync.dma_start(dbg["d_samsb"][:],
                              samsb[:].rearrange("p t q -> p (t q)"))
            nc.sync.dma_start(dbg["d_xga"][:],
                              xgA[:].rearrange("p s r w -> p (s r w)"))

    nc.compile()
    return nc


# --------------------------------------------------------------- entrypoint
def _get_program():
    if "nc" not in _CACHE:
        _CACHE["nc"] = _build_program()
    return _CACHE["nc"]


def _run(inputs, trace):
    inputs = {k: np.asarray(v) for k, v in inputs.items()}
    consts = _host_consts(inputs)
    nc = _get_program()
    x = inputs["x"].reshape(B, CIN, NPIX).astype(np.float16)
    in_maps = []
    for b in range(B):
        m = {k: v for k, v in consts.items()}
        m["xin"] = x[b]
        in_maps.append(m)
    res = run_bass_kernel_spmd(nc, in_maps, list(range(B)), trace=trace)
    out = np.stack([res.results[i]["out"] for i in range(B)])
    out = out.reshape(B, CIN, H, W).astype(np.float32)
    out += inputs["x"].astype(np.float32)  # residual in f32 on host
    return out, res


def kernel(**inputs):
    return _run(inputs, False)[0]


def kernel_traced(**inputs):
    """Like kernel() but with NTFF tracing; returns (out, BassKernelResults)."""
    return _run(inputs, True)


# revision 79
# speedup vs baseline: 1.0177x; 1.0177x over previous
"""Trainium2 Bass kernel for nn_Bottleneck_57561151701110 (SAM pairwise
bottleneck block). Data-parallel over batch: 8 images -> 8 NeuronCores.

v3 (~105us vs 121us v2 baseline):
 - fp16 end-to-end: x in / out in fp16 (HBM traffic halved); the +x
   residual is added on the host in f32 (also improves accuracy).
 - conv1/conv2 computed directly in band layout via PE column-tiling
   (tile_position col groups), killing the x12s band-remap DMAs; the
   x2bB shifted copy becomes a second eviction of the same PSUM.
 - xgB eliminated: dw=0 aggregation planes multiply the full 58-wide
   xgA rows against a column-shifted copy of e (esh, one extra ACT
   write per dw=0 k); the identity matmul reads the odd-offset window
   (PE has no alignment constraint), so every DVE multiply stays in
   2x mode with no shifted xg copy and no DMA chain.
 - PE warmup runs on a DVE-memset tile with no DMA dependency, so HAM
   unthrottles to 2.4 GHz before conv1 (K=8/8 from ~10us).
 - evictions balanced: conv3 t=0 ACT / t=1 DVE; hp fully DVE; e/esh
   on ACT (its exp chain paces phase C).
 - sam remaps issue per s-quad (samsb channel order permuted so each
   quad's 64 dst partitions are contiguous; convo lhsT rows permuted
   to match), and the last quad's issues split across both HWDGE
   queues -- the tail remap latency roughly halves.

Per-core pipeline (one 256x56x56 image):
  conv1/conv2 (PE fp16, 4-band col-tiles) -> x1b/x2b/x2bB (ACT evict) ->
  conv3 (PE) -> x3sf fp16 (58x58 zero-border planes) -> xgA remaps
  per k: fr = relu(x1b - x2win) (DVE) -> w1/pos/w2 blockdiag matmuls
    (PE) -> hp relu (DVE) / e exp + esh (ACT) -> Z (PE identity)
  rz = exp(-ln(Z)) (ACT) -> agg: pk = xg*e (DVE), k-sum (PE identity),
  Prelu (ACT), *rz (DVE) -> sqf -> sam remap (DMA) -> convo (PE) ->
  Prelu+bias (ACT) -> out fp16 -> host adds +x in f32.
"""

import os
import sys

for _p in ("/opt/trn_rl_repo", os.path.expanduser("~/.axon_site/_ro/trn_rl_repo")):
    if os.path.isdir(_p) and _p not in sys.path:
        sys.path.insert(0, _p)

from contextlib import ExitStack

import numpy as np

import concourse.bass as bass
import concourse.bacc as bacc
import concourse.tile as tile
from concourse import mybir
from concourse.bass_utils import run_bass_kernel_spmd

dt = mybir.dt
ALU = mybir.AluOpType
ACTF = mybir.ActivationFunctionType

B, CIN, H, W = 8, 256, 56, 56
NPIX = H * W            # 3136
REL, MID, OUT = 32, 256, 256
SHARE = 8
NB = 4                  # row bands
BH = H // NB            # 14 rows per band
Q = BH * W              # 784 band pixels
NEG = 0.01
BN_EPS = 1e-5
CCH = 448               # conv free chunk (8 rows)
NCH = NPIX // CCH       # 7
OFFS = [(dh, dw) for dh in (-1, 0, 1) for dw in (-1, 0, 1)]
# phase-C/E processing order: dh=0 first (no edge-row dependency in x2b),
# dw=0 offsets last (they need x2bB / the shifted-e copies)
OFFS_ORD = [3, 5, 0, 2, 6, 8, 1, 4, 7]
NSQ = SHARE // 2        # 4 s per quad
HBW = Q // 2            # 392 half-band pixels (7 rows)

F32, BF16 = dt.float32, dt.float16  # 16-bit = fp16 (values < 16)

_CACHE = {}

# packed const layouts (cols)
#  cb (fp16, 128 part): c1T (2*32) | c2T (2*32) | c3 (2*256) | co (2*256)
#                        | w1 (128) | w2 (128) | ident (128)
CB_C1, CB_C2, CB_C3, CB_CO = 0, 64, 128, 640
CB_W1, CB_W2, CB_ID = 1152, 1280, 1408
CB_COLS = 1536
#  cf (f32, 128 part): 8 bias vecs
#   0: b1 (tiled)  1: b2 (tiled)  2: conv3_b[:128]  3: conv3_b[128:]
#   4: betab (tiled)  5: w2_b (tiled)  6: biaso[:128]  7: biaso[128:]
CF_COLS = 8
#  pb (fp16, 8 part): post (128) | posr (9*784)
PB_POST, PB_POSR = 0, 128
PB_COLS = 128 + 9 * Q

# band b needs x rows 14b-1..14b+15 -> conv chunks 0..band_last_chunk[b]
BAND_LAST_CHUNK = [1, 3, 5, 6]
# conv3 runs in 4 chunks of 14 rows; band b needs chunks 0..BAND3_LAST[b]
BAND3_LAST = [1, 2, 3, 3]


# ----------------------------------------------------------------- host prep
def _position(h, w):
    loc_w = np.tile(np.linspace(-1.0, 1.0, w, dtype=np.float32)[None, :], (h, 1))
    loc_h = np.tile(np.linspace(-1.0, 1.0, h, dtype=np.float32)[:, None], (1, w))
    return np.stack([loc_w, loc_h], axis=0)  # (2, H, W)


def _host_consts(inp):
    f32 = np.float32
    f16 = np.float16
    inv_a = (inp["bna_g"] / np.sqrt(inp["bna_v"] + BN_EPS)).astype(f32)
    beta_a = (inp["bna_b"] - inp["bna_m"] * inv_a).astype(f32)
    inv_b = (inp["bnb_g"] / np.sqrt(inp["bnb_v"] + BN_EPS)).astype(f32)
    beta_b = (inp["bnb_b"] - inp["bnb_m"] * inv_b).astype(f32)

    w1c = inp["conv1_w"] * inv_a[:REL, None]          # (32, 256)
    b1 = inp["conv1_b"] * inv_a[:REL] + beta_a[:REL]
    w2c = inp["conv2_w"] * inv_a[:REL, None]
    b2 = inp["conv2_b"] * inv_a[:REL]

    # lhsT chunks: [p, t, o] = w[o, 128t+p]
    c1T = np.transpose(w1c.T.reshape(2, 128, REL), (1, 0, 2))   # (128, 2, 32)
    c2T = np.transpose(w2c.T.reshape(2, 128, REL), (1, 0, 2))
    c3 = np.transpose(inp["conv3_w"].T.reshape(2, 128, MID), (1, 0, 2))
    # convo lhsT rows permuted to match the per-s-quad sam remap layout:
    # samsb partition p' = 64*(s//4) + 4*g + (s%4) holds channel
    # c = 128t + 8g + s  (g = group-within-half, s = share index)
    perm = np.array([8 * ((p % 64) // 4) + 4 * (p // 64) + (p % 4)
                     for p in range(128)])
    cw = inp["convo_w"]
    co = np.zeros((128, 2, OUT), np.float32)
    for tk in range(2):
        co[:, tk, :] = cw[:, 128 * tk + perm].T

    # W1' with bnb scale folded
    w1p = (inp["w1"] * inv_b[:, None]).astype(f32)  # (32, 34)
    w1a, w1b = w1p[:, :REL], w1p[:, REL:]
    lhsT_w1 = np.zeros((128, 128), f32)
    lhsT_pos = np.zeros((8, 128), f32)
    lhsT_w2 = np.zeros((128, 128), f32)
    for b in range(NB):
        lhsT_w1[32 * b:32 * b + 32, 32 * b:32 * b + 32] = w1a.T  # [c, o]
        lhsT_pos[2 * b:2 * b + 2, 32 * b:32 * b + 32] = w1b.T    # [c2, o]
        lhsT_w2[32 * b:32 * b + 32, 32 * b:32 * b + 32] = inp["w2"].T  # [o, g]

    # position branch, batch independent: posr[2b+c2, 784k+q] =
    # relu(inv_a[32+c2]*subp[c2,k,band b pix q] + beta_a[32+c2])
    pos = _position(H, W)
    pc = np.einsum("oc,chw->ohw", inp["convp_w"], pos) + inp["convp_b"][:, None, None]
    pcp = np.pad(pc, ((0, 0), (1, 1), (1, 1)))
    posr = np.zeros((8, 9 * Q), f32)
    for k, (dh, dw) in enumerate(OFFS):
        sub = pc - pcp[:, 1 + dh:1 + dh + H, 1 + dw:1 + dw + W]  # (2,56,56)
        v = np.maximum(inv_a[REL:, None, None] * sub + beta_a[REL:, None, None], 0.0)
        vb = v.reshape(2, NB, BH, W)  # (c2, b, r, w)
        for b in range(NB):
            posr[2 * b:2 * b + 2, Q * k:Q * (k + 1)] = vb[:, b].reshape(2, Q)

    vecs = np.zeros((128, CF_COLS), f32)
    vecs[:, 0] = np.tile(b1, NB)
    vecs[:, 1] = np.tile(b2, NB)
    vecs[:, 2] = inp["conv3_b"][:128]
    vecs[:, 3] = inp["conv3_b"][128:]
    vecs[:, 4] = np.tile(beta_b, NB)
    vecs[:, 5] = np.tile(inp["w2_b"], NB)
    vecs[:, 6] = inp["convo_b"][:128]
    vecs[:, 7] = inp["convo_b"][128:]

    cb = np.zeros((128, CB_COLS), f16)
    cb[:, CB_C1:CB_C1 + 64] = c1T.reshape(128, 64)
    cb[:, CB_C2:CB_C2 + 64] = c2T.reshape(128, 64)
    cb[:, CB_C3:CB_C3 + 512] = c3.reshape(128, 512)
    cb[:, CB_CO:CB_CO + 512] = co.reshape(128, 512)
    cb[:, CB_W1:CB_W1 + 128] = lhsT_w1.astype(f16)
    cb[:, CB_W2:CB_W2 + 128] = lhsT_w2.astype(f16)
    cb[:, CB_ID:CB_ID + 128] = np.eye(128, dtype=f16)

    pb = np.zeros((8, PB_COLS), f16)
    pb[:, PB_POST:PB_POST + 128] = lhsT_pos.astype(f16)
    pb[:, PB_POSR:] = posr.astype(f16)

    return {"cf": vecs, "cb": cb, "pb": pb}


DEBUG = os.environ.get("KDEBUG", "0") == "1"


# ------------------------------------------------------------ program build
def _build_program():
    nc = bacc.Bacc("TRN2", target_bir_lowering=False, debug=False,
                   enable_asserts=False, num_devices=8)

    xind = nc.dram_tensor("xin", [CIN, NPIX], BF16, kind="ExternalInput").ap()
    cfd = nc.dram_tensor("cf", [128, CF_COLS], F32, kind="ExternalInput").ap()
    cbd = nc.dram_tensor("cb", [128, CB_COLS], BF16, kind="ExternalInput").ap()
    pbd = nc.dram_tensor("pb", [8, PB_COLS], BF16, kind="ExternalInput").ap()
    outd = nc.dram_tensor("out", [CIN, NPIX], BF16, kind="ExternalOutput").ap()
    dbg = {}
    if DEBUG:
        for nm, shape in [("d_x1b", [128, Q]), ("d_x2b", [128, 16 * 58]),
                          ("d_x2bB", [128, 16 * 58]),
                          ("d_x3sf", [128, 2 * 58 * 58]),
                          ("d_e0", [128, Q]), ("d_e4", [128, Q]),
                          ("d_rz", [128, Q]), ("d_sqf", [128, SHARE * Q]),
                          ("d_samsb", [128, 2 * NPIX]),
                          ("d_xga", [128, SHARE * 16 * 58]),
                          ("d_xgb", [128, SHARE * 16 * 58])]:
            dbg[nm] = nc.dram_tensor(nm, shape, BF16, kind="ExternalOutput").ap()

    with tile.TileContext(nc) as tc, ExitStack() as ctx:
        nc_ = tc.nc

        # ---- persistent pools
        cpool = ctx.enter_context(tc.tile_pool(name="consts", bufs=1))
        xpool = ctx.enter_context(tc.tile_pool(name="xin", bufs=1))
        sbp = ctx.enter_context(tc.tile_pool(name="sbp", bufs=1))
        epool = ctx.enter_context(tc.tile_pool(name="epool", bufs=9))

        # ---- const loads: 3 packed DMAs on the scalar queue
        cbt = cpool.tile([128, CB_COLS], BF16, tag="cb")
        nc.scalar.dma_start(cbt[:], cbd[:])
        cft = cpool.tile([128, CF_COLS], F32, tag="cf")
        nc.scalar.dma_start(cft[:], cfd[:])
        pbt = cpool.tile([8, PB_COLS], BF16, tag="pb")
        nc.scalar.dma_start(pbt[:], pbd[:])

        c1t = cbt[:, CB_C1:CB_C1 + 64].rearrange("p (t o) -> p t o", t=2)
        c2t = cbt[:, CB_C2:CB_C2 + 64].rearrange("p (t o) -> p t o", t=2)
        c3t = cbt[:, CB_C3:CB_C3 + 512].rearrange("p (t o) -> p t o", t=2)
        cot = cbt[:, CB_CO:CB_CO + 512].rearrange("p (t o) -> p t o", t=2)
        w1t = cbt[:, CB_W1:CB_W1 + 128]
        w2t = cbt[:, CB_W2:CB_W2 + 128]
        identt = cbt[:, CB_ID:CB_ID + 128]
        vecst = cft
        post = pbt[:, PB_POST:PB_POST + 128]
        prt = pbt[:, PB_POSR:].rearrange("p (k q) -> p k q", k=9)

        # ---- x input: one resident fp16 tile, 7 chunk DMAs (sync queue)
        xfull = xpool.tile([128, 2, NPIX], BF16, tag="xfull")
        xin_r = xind[:].rearrange("(t p) q -> p t q", t=2)
        for c in range(NCH):
            sl = slice(CCH * c, CCH * (c + 1))
            nc.sync.dma_start(xfull[:, :, sl], xin_r[:, :, sl])

        # ---- band-layout tiles + zero borders (memsets on idle GpSimd)
        x1b = sbp.tile([128, Q], BF16, tag="x1b")
        x2b = sbp.tile([128, 16, 58], BF16, tag="x2b")
        x2bB = sbp.tile([128, 16, 58], BF16, tag="x2bB")
        x3sf = sbp.tile([128, 2, 58, 58], BF16, tag="x3sf")
        xgA = sbp.tile([128, SHARE, 16, 58], BF16, tag="xgA")
        nc_.gpsimd.memset(x2b[:], 0.0)
        nc_.gpsimd.memset(x2bB[:], 0.0)
        for t in range(2):
            nc_.gpsimd.memset(x3sf[:, t, 0:1, :], 0.0)      # top border row
            nc_.gpsimd.memset(x3sf[:, t, 57:58, :], 0.0)    # bottom border row
            nc_.gpsimd.memset(x3sf[:, t, 1:57, 0:1], 0.0)   # left border col
            nc_.gpsimd.memset(x3sf[:, t, 1:57, 57:58], 0.0)  # right border col

        # ---- PE warmup on UNINITIALIZED SBUF: zero dependencies, so the
        # matmuls start the moment the PE engine boots (garbage/NaN results
        # land in a dummy PSUM bank that is never read). ~6.4us of activity
        # bridges the input-DMA wait and unthrottles HAM to 2.4 GHz before
        # conv1 starts.
        wpool = ctx.enter_context(tc.tile_pool(name="wpool", bufs=1))
        wsrc = wpool.tile([128, 128], BF16, tag="wsrc")
        nc_.vector.memset(wsrc[:], 0.0)   # DVE is idle at boot
        wexp = wpool.tile([1, 8], F32, tag="wexp")
        nc_.vector.memset(wexp[:], 0.0)
        wdst = wpool.tile([1, 8], F32, tag="wdst")
        nc_.scalar.activation(wdst[:], wexp[:], ACTF.Exp)  # exp table warm
        with tc.tile_pool(name="pwarm", bufs=1, space="PSUM") as pwarm:
            wps_ = pwarm.tile([128, 128], F32, tag="wps")
            for _ in range(60):
                nc_.tensor.matmul(wps_[:], wsrc[:], wsrc[:],
                                  start=True, stop=True)

        # ---- phase A: conv1+conv2 directly in band layout via col-tiling.
        # x1p[32b+j, q]  = conv1 out channel j at band-b pixel q
        # x2p[32b+j, r*56+w] = conv2 out channel j at img (14b-1+r, w)
        pscope0 = ExitStack()
        pp12 = pscope0.enter_context(tc.tile_pool(name="pp12", bufs=1, space="PSUM"))
        x1p = pp12.tile([128, Q], F32, tag="x1p")
        x2p = pp12.tile([128, 16 * W], F32, tag="x2p")
        for b in range(NB):
            bsl = slice(32 * b, 32 * b + 32)
            tp = (0, 32 * b)
            # conv1: band pixels 784b..784(b+1), bank-safe col splits
            for (c0, cw) in ((0, 512), (512, Q - 512)):
                for t in range(2):
                    nc_.tensor.matmul(
                        x1p[bsl, c0:c0 + cw], c1t[:, t, :],
                        xfull[:, t, Q * b + c0:Q * b + c0 + cw],
                        start=(t == 0), stop=(t == 1), tile_position=tp)
            # conv2: rows 14b-1..14b+15 (halo), invalid edge rows skipped
            p0 = (BH * b - 1) * W            # img pixel of dst col 0
            dlo = W if b == 0 else 0         # first valid dst col
            dhi = 16 * W - W if b == NB - 1 else 16 * W
            splits = [(d0, d1) for (d0, d1) in ((dlo, 512), (512, dhi))
                      if d1 > d0]
            for (d0, d1) in splits:
                for t in range(2):
                    nc_.tensor.matmul(
                        x2p[bsl, d0:d1], c2t[:, t, :],
                        xfull[:, t, p0 + d0:p0 + d1],
                        start=(t == 0), stop=(t == 1), tile_position=tp)
        # evictions (ACT, fp16, bias folded): x1b flat; x2b/x2bB as 16x58
        # zero-border planes, data at col offset 1 (x2b) / 2 (x2bB)
        nc_.scalar.activation(x1b[:], x1p[:], ACTF.Identity, bias=vecst[:, 0:1])

        def evict_x2(xt, co_, eng):
            if eng is nc_.scalar:
                eng.activation(xt[:, 1:15, co_:co_ + W],
                               x2p[:, W:15 * W].rearrange(
                                   "p (r w) -> p r w", w=W),
                               ACTF.Identity, bias=vecst[:, 1:2])
            else:
                eng.tensor_scalar(xt[:, 1:15, co_:co_ + W],
                                  x2p[:, W:15 * W].rearrange(
                                      "p (r w) -> p r w", w=W),
                                  vecst[:, 1:2], None, op0=ALU.add)
            # edge rows. row 15 (valid for bands 0..2) is one op at base
            # partition 0; row 0 (bands 1..3) must be split per band (PSUM
            # APs at partition offset p>0 may not span more than 32
            # partitions)
            edges = [(slice(0, 96), 15, slice(15 * W, 16 * W))] + \
                    [(slice(32 * b, 32 * b + 32), 0, slice(0, W))
                     for b in range(1, NB)]
            for (bsl, r, ssl) in edges:
                if eng is nc_.scalar:
                    eng.activation(xt[bsl, r:r + 1, co_:co_ + W],
                                   x2p[bsl, ssl].unsqueeze(1),
                                   ACTF.Identity, bias=vecst[bsl, 1:2])
                else:
                    eng.tensor_scalar(xt[bsl, r:r + 1, co_:co_ + W],
                                      x2p[bsl, ssl].unsqueeze(1),
                                      vecst[bsl, 1:2], None, op0=ALU.add)

        evict_x2(x2b, 1, nc_.scalar)
        evict_x2(x2bB, 2, nc_.scalar)
        pscope0.close()

        # ---- phase B: conv3 (evictions split ACT/DVE; xgA remaps per band)
        pscope1 = ExitStack()
        ppc = pscope1.enter_context(tc.tile_pool(name="ppc", bufs=2, space="PSUM"))
        for c in range(NCH):
            sl = slice(CCH * c, CCH * (c + 1))
            for t in range(2):
                ps3 = ppc.tile([128, CCH], F32, tag="conv")
                nc_.tensor.matmul(ps3[:], c3t[:, 0, 128 * t:128 * (t + 1)],
                                  xfull[:, 0, sl], start=True, stop=False)
                nc_.tensor.matmul(ps3[:], c3t[:, 1, 128 * t:128 * (t + 1)],
                                  xfull[:, 1, sl], start=False, stop=True)
                dst3 = x3sf[:, t, 1 + 8 * c:1 + 8 * c + 8, 1:57]
                src3 = ps3[:].rearrange("p (r w) -> p r w", w=W)
                if t == 0:
                    nc_.scalar.activation(dst3, src3, ACTF.Identity,
                                          bias=vecst[:, 2:3])
                else:
                    nc_.vector.tensor_scalar(dst3, src3, vecst[:, 3:4], None,
                                             op0=ALU.add)
            # xg band remaps: band b needs x3sf plane rows 14b..14b+16
            # xgA[32b+16t+g, s, r, w] = x3sf[8g+s, t, 14b+r, w]
            # (contiguous 928-elem runs per partition -> cheap descriptors)
            for b in range(NB):
                if BAND_LAST_CHUNK[b] == c:
                    for t in range(2):
                        psl = slice(32 * b + 16 * t, 32 * b + 16 * t + 16)
                        nc.sync.dma_start(
                            xgA[psl],
                            x3sf[:, t, 14 * b:14 * b + 16, :])
        pscope1.close()

        xgo = {-1: 0, 1: 2}   # xgA col offset of the dh-window, dw != 0
        x2 = {-1: x2b, 0: x2bB, 1: x2b}
        x2o = {-1: 0, 0: 2, 1: 2}

        # ---- phase C: per-k logits + exp + Z (PE fp16 + ACT)
        frpool = ctx.enter_context(tc.tile_pool(name="frpool", bufs=4))
        hpool = ctx.enter_context(tc.tile_pool(name="hpool", bufs=3))
        eshpool = ctx.enter_context(tc.tile_pool(name="eshpool", bufs=3))
        esh_d = {}
        pscope2 = ExitStack()
        ppz = pscope2.enter_context(tc.tile_pool(name="ppz", bufs=1, space="PSUM"))
        pscope3 = ExitStack()
        pph = pscope3.enter_context(tc.tile_pool(name="pph", bufs=3, space="PSUM"))
        ek = {}
        zps = ppz.tile([128, 1024], F32, tag="zps")
        wsl = [slice(0, 512), slice(512, Q)]
        # software-pipelined: iteration i emits stage-1 work for k_i and
        # stage-2 work for k_{i-1}, so no engine FIFO head-of-line blocks.
        hp_d = {}
        for i in range(10):
            if i < 9:
                k = OFFS_ORD[i]
                dh, dw = OFFS[k]
                co_ = x2o[dw]
                fs = frpool.tile([128, BH, W], BF16, tag="fs")
                nc_.vector.tensor_tensor(
                    fs[:], x1b[:].rearrange("p (r w) -> p r w", w=W),
                    x2[dw][:, 1 + dh:1 + dh + BH, co_:co_ + W],
                    ALU.subtract)
                fr = frpool.tile([128, Q], BF16, tag="fr")
                nc_.vector.tensor_scalar(fr[:].rearrange("p (r w) -> p r w", w=W),
                                         fs[:], 0.0, None, op0=ALU.max)
                hps = pph.tile([128, 1024], F32, tag="hw")
                # same-stationary matmuls back-to-back (one weight load/pair)
                for s in wsl:
                    nc_.tensor.matmul(hps[:, s], w1t[:], fr[:, s],
                                      start=True, stop=False)
                for s in wsl:
                    nc_.tensor.matmul(hps[:, s], post[:], prt[0:8, k, s],
                                      start=False, stop=True)
                hp = hpool.tile([128, Q], BF16, tag="hp")
                hp_d[k] = hp
                # hp eviction fully on DVE: ACT is the phase-C critical
                # chain (it owns all the exp evictions)
                nc_.vector.tensor_scalar(hp[:], hps[:, 0:Q],
                                         vecst[:, 4:5], 0.0,
                                         op0=ALU.add, op1=ALU.max)
            if i >= 1:
                kp = OFFS_ORD[i - 1]
                wps = pph.tile([128, 1024], F32, tag="hw")
                for s in wsl:
                    nc_.tensor.matmul(wps[:, s], w2t[:], hp_d[kp][:, s],
                                      start=True, stop=True)
                e = epool.tile([128, Q], BF16, tag="e")
                nc_.scalar.activation(e[:], wps[:, 0:Q], ACTF.Exp,
                                      bias=vecst[:, 5:6])
                if OFFS[kp][1] == 0:
                    # dw=0 planes also need e at odd column parity: a second
                    # ACT write into a 58-wide row layout at col offset 1
                    # (esh[g, r, 1+w] = e[g, r, w]; cols 0/57 stay garbage,
                    # never read)
                    esh = eshpool.tile([128, BH, 58], BF16, tag="esh")
                    esh_d[kp] = esh
                    nc_.scalar.activation(
                        esh[:, :, 1:57],
                        wps[:, 0:Q].rearrange("p (r w) -> p r w", w=W),
                        ACTF.Exp, bias=vecst[:, 5:6])
                for s in wsl:
                    nc_.tensor.matmul(zps[:, s], identt[:], e[:, s],
                                      start=(i == 1), stop=(i == 9))
                ek[kp] = e
        pscope3.close()

        # ---- phase E: aggregation + convo, per half-band row group qp.
        # The rz = exp(-ln(Z)) computation (and its ACT table swaps) is
        # emitted AFTER the first quad's multiply/matmul rounds so the two
        # table loads hide under quad-0 compute.
        rzpool = ctx.enter_context(tc.tile_pool(name="rzpool", bufs=1))
        lnz = rzpool.tile([128, Q], F32, tag="lnz")
        rz16 = rzpool.tile([128, Q], BF16, tag="rz16")
        pkpool = ctx.enter_context(tc.tile_pool(name="pkpool", bufs=6))
        sqf = sbp.tile([128, SHARE, Q], BF16, tag="sqf")
        lkpool = ctx.enter_context(tc.tile_pool(name="lkpool", bufs=3))
        opool = ctx.enter_context(tc.tile_pool(name="opool", bufs=4))
        samsb = sbp.tile([128, 2, NPIX], BF16, tag="samsb")
        pscope4 = ExitStack()
        pps = pscope4.enter_context(tc.tile_pool(name="pps", bufs=1, space="PSUM"))
        ppo = pscope4.enter_context(tc.tile_pool(name="ppo", bufs=2, space="PSUM"))

        first_quad = [True]

        def emit_pk(k, qp, sq, engine):
            # pk plane for offset k, quad (qp, sq); returns matmul rhs list
            dh, dw = OFFS[k]
            r0 = 1 + dh + 7 * qp
            sqs = slice(NSQ * sq, NSQ * (sq + 1))
            if dw != 0:
                pk = pkpool.tile([128, NSQ, 7, W], BF16, tag="pk")
                co_ = xgo[dw]
                engine.tensor_tensor(
                    pk[:],
                    xgA[:, sqs, r0:r0 + 7, co_:co_ + W],
                    ek[k][:].rearrange("p (r w) -> p r w", w=W)
                    [:, 7 * qp:7 * qp + 7, :].unsqueeze(1)
                    .broadcast_to((128, NSQ, 7, W)),
                    ALU.mult)
                return [pk[:, c4, :, :] for c4 in range(4)]
            # dw=0: full 58-wide product against the shifted-e copy (both
            # operands at even parity); the identity matmul reads the
            # odd-offset window instead
            pk = pkpool.tile([128, NSQ, 7, 58], BF16, tag="pk8")
            engine.tensor_tensor(
                pk[:],
                xgA[:, sqs, r0:r0 + 7, 0:58],
                esh_d[k][:, 7 * qp:7 * qp + 7, :].unsqueeze(1)
                .broadcast_to((128, NSQ, 7, 58)),
                ALU.mult)
            return [pk[:, c4, :, 1:57] for c4 in range(4)]

        for qp in range(2):
            for sq in range(2):
                sam = pps.tile([128, 2048], F32, tag="sam")
                samv = sam[:].rearrange("p (a j) -> p a j", j=512)[:, :, 0:HBW]
                for j, k in enumerate(OFFS_ORD):
                    rhs = emit_pk(k, qp, sq, nc_.vector)
                    for c4 in range(4):
                        nc_.tensor.matmul(
                            sam[:, 512 * c4:512 * c4 + HBW], identt[:],
                            rhs[c4],
                            start=(j == 0), stop=(j == 8))
                if first_quad[0]:
                    # rz = exp(-ln(Z)): table loads overlap quad-0 compute
                    nc_.scalar.activation(lnz[:], zps[:, 0:Q], ACTF.Ln)
                    nc_.scalar.activation(rz16[:], lnz[:], ACTF.Exp,
                                          scale=-1.0)
                    first_quad[0] = False
                # leaky(sam) then * rz -> sqf quad slice
                lk = lkpool.tile([128, NSQ, HBW], BF16, tag="lk")
                nc_.scalar.activation(lk[:], samv, ACTF.Prelu, alpha=NEG)
                nc_.vector.tensor_tensor(
                    sqf[:, NSQ * sq:NSQ * (sq + 1), HBW * qp:HBW * (qp + 1)],
                    lk[:],
                    rz16[:, HBW * qp:HBW * (qp + 1)].unsqueeze(1)
                    .broadcast_to((128, NSQ, HBW)),
                    ALU.mult)
                # sam remap for this s-quad: dst partitions 64sq..64sq+64
                # are contiguous (convo lhsT rows permuted to match), so the
                # transfer starts one quad earlier than a full-s remap.
                # dst [64, 392] pairs with src [16, 4, 392] in flat order:
                # dst 64sq+4g+smod <- sqf[32b+16t+g, 4sq+smod]
                for t in range(2):
                    for b in range(NB):
                        # the last quad's issues split across both HWDGE
                        # queues: ACT has no independent work left there,
                        # so head-of-line blocking is harmless and the
                        # 8x ~0.7us issue serialization halves
                        q_ = (nc.scalar if qp == 1 and sq == 1
                              and (t + b) % 2 == 1 else nc.sync)
                        q_.dma_start(
                            samsb[64 * sq:64 * sq + 64, t,
                                  Q * b + HBW * qp:Q * b + HBW * (qp + 1)],
                            sqf[32 * b + 16 * t:32 * b + 16 * t + 16,
                                NSQ * sq:NSQ * (sq + 1),
                                HBW * qp:HBW * (qp + 1)])
            # convo on this qp's half-bands; one merged store per band
            # (residual +x is added on the host in f32)
            for b in range(NB):
                po = Q * b + HBW * qp
                o = opool.tile([128, 2, HBW], BF16, tag="o")
                for to in range(2):
                    pso = ppo.tile([128, HBW], F32, tag="pso")
                    nc_.tensor.matmul(pso[:], cot[:, 0, 128 * to:128 * (to + 1)],
                                      samsb[:, 0, po:po + HBW],
                                      start=True, stop=False)
                    nc_.tensor.matmul(pso[:], cot[:, 1, 128 * to:128 * (to + 1)],
                                      samsb[:, 1, po:po + HBW],
                                      start=False, stop=True)
                    nc_.scalar.activation(o[:, to, :], pso[:], ACTF.Prelu,
                                          bias=vecst[:, 6 + to:7 + to],
                                          alpha=NEG)
                nc.sync.dma_start(
                    outd[:].rearrange("(t p) q -> p t q", t=2)
                    [:, :, po:po + HBW], o[:])
        pscope4.close()
        pscope2.close()

        if DEBUG:
            nc.sync.dma_start(dbg["d_x1b"][:], x1b[:])
            nc.sync.dma_start(dbg["d_x2b"][:],
                              x2b[:].rearrange("p r w -> p (r w)"))
            nc.sync.dma_start(dbg["d_x2bB"][:],
                              x2bB[:].rearrange("p r w -> p (r w)"))
            nc.sync.dma_start(dbg["d_x3sf"][:],
                              x3sf[:].rearrange("p t r w -> p (t r w)"))
            nc.sync.dma_start(dbg["d_e0"][:], ek[0][:])
            nc.sync.dma_start(dbg["d_e4"][:], ek[4][:])
            nc.sync.dma_start(dbg["d_rz"][:], rz16[:])
            nc.sync.dma_start(dbg["d_sqf"][:],
                              sqf[:].rearrange("p s q -> p (s q)"))
            nc.sync.dma_start(dbg["d_samsb"][:],
                              samsb[:].rearrange("p t q -> p (t q)"))
            nc.sync.dma_start(dbg["d_xga"][:],
                              xgA[:].rearrange("p s r w -> p (s r w)"))

    nc.compile()
    return nc


# --------------------------------------------------------------- entrypoint
def _get_program():
    if "nc" not in _CACHE:
        _CACHE["nc"] = _build_program()
    return _CACHE["nc"]


def _run(inputs, trace):
    inputs = {k: np.asarray(v) for k, v in inputs.items()}
    consts = _host_consts(inputs)
    nc = _get_program()
    x = inputs["x"].reshape(B, CIN, NPIX).astype(np.float16)
    in_maps = []
    for b in range(B):
        m = {k: v for k, v in consts.items()}
        m["xin"] = x[b]
        in_maps.append(m)
    res = run_bass_kernel_spmd(nc, in_maps, list(range(B)), trace=trace)
    out = np.stack([res.results[i]["out"] for i in range(B)])
    out = out.reshape(B, CIN, H, W).astype(np.float32)
    out += inputs["x"].astype(np.float32)  # residual in f32 on host
    return out, res


def kernel(**inputs):
    return _run(inputs, False)[0]


def kernel_traced(**inputs):
    """Like kernel() but with NTFF tracing; returns (out, BassKernelResults)."""
    return _run(inputs, True)


# revision 80
# speedup vs baseline: 1.0366x; 1.0186x over previous
"""Trainium2 Bass kernel for nn_Bottleneck_57561151701110 (SAM pairwise
bottleneck block). Data-parallel over batch: 8 images -> 8 NeuronCores.

v3 (~105us vs 121us v2 baseline):
 - fp16 end-to-end: x in / out in fp16 (HBM traffic halved); the +x
   residual is added on the host in f32 (also improves accuracy).
 - conv1/conv2 computed directly in band layout via PE column-tiling
   (tile_position col groups), killing the x12s band-remap DMAs; the
   x2bB shifted copy becomes a second eviction of the same PSUM.
 - xgB eliminated: dw=0 aggregation planes multiply the full 58-wide
   xgA rows against a column-shifted copy of e (esh, one extra ACT
   write per dw=0 k); the identity matmul reads the odd-offset window
   (PE has no alignment constraint), so every DVE multiply stays in
   2x mode with no shifted xg copy and no DMA chain.
 - PE warmup runs on a DVE-memset tile with no DMA dependency, so HAM
   unthrottles to 2.4 GHz before conv1 (K=8/8 from ~10us).
 - evictions balanced: conv3 t=0 ACT / t=1 DVE; hp fully DVE; e/esh
   on ACT (its exp chain paces phase C).
 - sam remaps issue per s-quad (samsb channel order permuted so each
   quad's 64 dst partitions are contiguous; convo lhsT rows permuted
   to match), and the last quad's issues split across both HWDGE
   queues -- the tail remap latency roughly halves.

Per-core pipeline (one 256x56x56 image):
  conv1/conv2 (PE fp16, 4-band col-tiles) -> x1b/x2b/x2bB (ACT evict) ->
  conv3 (PE) -> x3sf fp16 (58x58 zero-border planes) -> xgA remaps
  per k: fr = relu(x1b - x2win) (DVE) -> w1/pos/w2 blockdiag matmuls
    (PE) -> hp relu (DVE) / e exp + esh (ACT) -> Z (PE identity)
  rz = exp(-ln(Z)) (ACT) -> agg: pk = xg*e (DVE), k-sum (PE identity),
  Prelu (ACT), *rz (DVE) -> sqf -> sam remap (DMA) -> convo (PE) ->
  Prelu+bias (ACT) -> out fp16 -> host adds +x in f32.
"""

import os
import sys

for _p in ("/opt/trn_rl_repo", os.path.expanduser("~/.axon_site/_ro/trn_rl_repo")):
    if os.path.isdir(_p) and _p not in sys.path:
        sys.path.insert(0, _p)

from contextlib import ExitStack

import numpy as np

import concourse.bass as bass
import concourse.bacc as bacc
import concourse.tile as tile
from concourse import mybir
from concourse.bass_utils import run_bass_kernel_spmd

dt = mybir.dt
ALU = mybir.AluOpType
ACTF = mybir.ActivationFunctionType

B, CIN, H, W = 8, 256, 56, 56
NPIX = H * W            # 3136
REL, MID, OUT = 32, 256, 256
SHARE = 8
NB = 4                  # row bands
BH = H // NB            # 14 rows per band
Q = BH * W              # 784 band pixels
NEG = 0.01
BN_EPS = 1e-5
CCH = 448               # conv free chunk (8 rows)
NCH = NPIX // CCH       # 7
OFFS = [(dh, dw) for dh in (-1, 0, 1) for dw in (-1, 0, 1)]
# phase-C/E processing order: dh=0 first (no edge-row dependency in x2b),
# dw=0 offsets last (they need x2bB / the shifted-e copies)
OFFS_ORD = [3, 5, 0, 2, 6, 8, 1, 4, 7]
NSQ = SHARE // 2        # 4 s per quad
HBW = Q // 2            # 392 half-band pixels (7 rows)

F32, BF16 = dt.float32, dt.float16  # 16-bit = fp16 (values < 16)

_CACHE = {}

# packed const layouts (cols)
#  cb (fp16, 128 part): c1T (2*32) | c2T (2*32) | c3 (2*256) | co (2*256)
#                        | w1 (128) | w2 (128) | ident (128)
CB_C1, CB_C2, CB_C3, CB_CO = 0, 64, 128, 640
CB_W1, CB_W2, CB_ID = 1152, 1280, 1408
CB_COLS = 1536
#  cf (f32, 128 part): 8 bias vecs
#   0: b1 (tiled)  1: b2 (tiled)  2: conv3_b[:128]  3: conv3_b[128:]
#   4: betab (tiled)  5: w2_b (tiled)  6: biaso[:128]  7: biaso[128:]
CF_COLS = 8
#  pb (fp16, 8 part): post (128) | posr (9*784)
PB_POST, PB_POSR = 0, 128
PB_COLS = 128 + 9 * Q

# band b needs x rows 14b-1..14b+15 -> conv chunks 0..band_last_chunk[b]
BAND_LAST_CHUNK = [1, 3, 5, 6]
# conv3 runs in 4 chunks of 14 rows; band b needs chunks 0..BAND3_LAST[b]
BAND3_LAST = [1, 2, 3, 3]


# ----------------------------------------------------------------- host prep
def _position(h, w):
    loc_w = np.tile(np.linspace(-1.0, 1.0, w, dtype=np.float32)[None, :], (h, 1))
    loc_h = np.tile(np.linspace(-1.0, 1.0, h, dtype=np.float32)[:, None], (1, w))
    return np.stack([loc_w, loc_h], axis=0)  # (2, H, W)


def _host_consts(inp):
    f32 = np.float32
    f16 = np.float16
    inv_a = (inp["bna_g"] / np.sqrt(inp["bna_v"] + BN_EPS)).astype(f32)
    beta_a = (inp["bna_b"] - inp["bna_m"] * inv_a).astype(f32)
    inv_b = (inp["bnb_g"] / np.sqrt(inp["bnb_v"] + BN_EPS)).astype(f32)
    beta_b = (inp["bnb_b"] - inp["bnb_m"] * inv_b).astype(f32)

    w1c = inp["conv1_w"] * inv_a[:REL, None]          # (32, 256)
    b1 = inp["conv1_b"] * inv_a[:REL] + beta_a[:REL]
    w2c = inp["conv2_w"] * inv_a[:REL, None]
    b2 = inp["conv2_b"] * inv_a[:REL]

    # lhsT chunks: [p, t, o] = w[o, 128t+p]
    c1T = np.transpose(w1c.T.reshape(2, 128, REL), (1, 0, 2))   # (128, 2, 32)
    c2T = np.transpose(w2c.T.reshape(2, 128, REL), (1, 0, 2))
    c3 = np.transpose(inp["conv3_w"].T.reshape(2, 128, MID), (1, 0, 2))
    # convo lhsT rows permuted to match the per-s-quad sam remap layout:
    # samsb partition p' = 64*(s//4) + 4*g + (s%4) holds channel
    # c = 128t + 8g + s  (g = group-within-half, s = share index)
    perm = np.array([8 * ((p % 64) // 4) + 4 * (p // 64) + (p % 4)
                     for p in range(128)])
    cw = inp["convo_w"]
    co = np.zeros((128, 2, OUT), np.float32)
    for tk in range(2):
        co[:, tk, :] = cw[:, 128 * tk + perm].T

    # W1' with bnb scale folded
    w1p = (inp["w1"] * inv_b[:, None]).astype(f32)  # (32, 34)
    w1a, w1b = w1p[:, :REL], w1p[:, REL:]
    lhsT_w1 = np.zeros((128, 128), f32)
    lhsT_pos = np.zeros((8, 128), f32)
    lhsT_w2 = np.zeros((128, 128), f32)
    for b in range(NB):
        lhsT_w1[32 * b:32 * b + 32, 32 * b:32 * b + 32] = w1a.T  # [c, o]
        lhsT_pos[2 * b:2 * b + 2, 32 * b:32 * b + 32] = w1b.T    # [c2, o]
        lhsT_w2[32 * b:32 * b + 32, 32 * b:32 * b + 32] = inp["w2"].T  # [o, g]

    # position branch, batch independent: posr[2b+c2, 784k+q] =
    # relu(inv_a[32+c2]*subp[c2,k,band b pix q] + beta_a[32+c2])
    pos = _position(H, W)
    pc = np.einsum("oc,chw->ohw", inp["convp_w"], pos) + inp["convp_b"][:, None, None]
    pcp = np.pad(pc, ((0, 0), (1, 1), (1, 1)))
    posr = np.zeros((8, 9 * Q), f32)
    for k, (dh, dw) in enumerate(OFFS):
        sub = pc - pcp[:, 1 + dh:1 + dh + H, 1 + dw:1 + dw + W]  # (2,56,56)
        v = np.maximum(inv_a[REL:, None, None] * sub + beta_a[REL:, None, None], 0.0)
        vb = v.reshape(2, NB, BH, W)  # (c2, b, r, w)
        for b in range(NB):
            posr[2 * b:2 * b + 2, Q * k:Q * (k + 1)] = vb[:, b].reshape(2, Q)

    vecs = np.zeros((128, CF_COLS), f32)
    vecs[:, 0] = np.tile(b1, NB)
    vecs[:, 1] = np.tile(b2, NB)
    vecs[:, 2] = inp["conv3_b"][:128]
    vecs[:, 3] = inp["conv3_b"][128:]
    vecs[:, 4] = np.tile(beta_b, NB)
    vecs[:, 5] = np.tile(inp["w2_b"], NB)
    vecs[:, 6] = inp["convo_b"][:128]
    vecs[:, 7] = inp["convo_b"][128:]

    cb = np.zeros((128, CB_COLS), f16)
    cb[:, CB_C1:CB_C1 + 64] = c1T.reshape(128, 64)
    cb[:, CB_C2:CB_C2 + 64] = c2T.reshape(128, 64)
    cb[:, CB_C3:CB_C3 + 512] = c3.reshape(128, 512)
    cb[:, CB_CO:CB_CO + 512] = co.reshape(128, 512)
    cb[:, CB_W1:CB_W1 + 128] = lhsT_w1.astype(f16)
    cb[:, CB_W2:CB_W2 + 128] = lhsT_w2.astype(f16)
    cb[:, CB_ID:CB_ID + 128] = np.eye(128, dtype=f16)

    pb = np.zeros((8, PB_COLS), f16)
    pb[:, PB_POST:PB_POST + 128] = lhsT_pos.astype(f16)
    pb[:, PB_POSR:] = posr.astype(f16)

    return {"cf": vecs, "cb": cb, "pb": pb}


DEBUG = os.environ.get("KDEBUG", "0") == "1"


# ------------------------------------------------------------ program build
def _build_program():
    nc = bacc.Bacc("TRN2", target_bir_lowering=False, debug=False,
                   enable_asserts=False, num_devices=8)

    xind = nc.dram_tensor("xin", [CIN, NPIX], BF16, kind="ExternalInput").ap()
    cfd = nc.dram_tensor("cf", [128, CF_COLS], F32, kind="ExternalInput").ap()
    cbd = nc.dram_tensor("cb", [128, CB_COLS], BF16, kind="ExternalInput").ap()
    pbd = nc.dram_tensor("pb", [8, PB_COLS], BF16, kind="ExternalInput").ap()
    outd = nc.dram_tensor("out", [CIN, NPIX], BF16, kind="ExternalOutput").ap()
    dbg = {}
    if DEBUG:
        for nm, shape in [("d_x1b", [128, Q]), ("d_x2b", [128, 16 * 58]),
                          ("d_x2bB", [128, 16 * 58]),
                          ("d_x3sf", [128, 2 * 58 * 58]),
                          ("d_e0", [128, Q]), ("d_e4", [128, Q]),
                          ("d_rz", [128, Q]), ("d_sqf", [128, SHARE * Q]),
                          ("d_samsb", [128, 2 * NPIX]),
                          ("d_xga", [128, SHARE * 16 * 58]),
                          ("d_xgb", [128, SHARE * 16 * 58])]:
            dbg[nm] = nc.dram_tensor(nm, shape, BF16, kind="ExternalOutput").ap()

    with tile.TileContext(nc) as tc, ExitStack() as ctx:
        nc_ = tc.nc

        # ---- persistent pools
        cpool = ctx.enter_context(tc.tile_pool(name="consts", bufs=1))
        xpool = ctx.enter_context(tc.tile_pool(name="xin", bufs=1))
        sbp = ctx.enter_context(tc.tile_pool(name="sbp", bufs=1))
        epool = ctx.enter_context(tc.tile_pool(name="epool", bufs=9))

        # ---- const loads: 3 packed DMAs on the scalar queue
        cbt = cpool.tile([128, CB_COLS], BF16, tag="cb")
        nc.scalar.dma_start(cbt[:], cbd[:])
        cft = cpool.tile([128, CF_COLS], F32, tag="cf")
        nc.scalar.dma_start(cft[:], cfd[:])
        pbt = cpool.tile([8, PB_COLS], BF16, tag="pb")
        nc.scalar.dma_start(pbt[:], pbd[:])

        c1t = cbt[:, CB_C1:CB_C1 + 64].rearrange("p (t o) -> p t o", t=2)
        c2t = cbt[:, CB_C2:CB_C2 + 64].rearrange("p (t o) -> p t o", t=2)
        c3t = cbt[:, CB_C3:CB_C3 + 512].rearrange("p (t o) -> p t o", t=2)
        cot = cbt[:, CB_CO:CB_CO + 512].rearrange("p (t o) -> p t o", t=2)
        w1t = cbt[:, CB_W1:CB_W1 + 128]
        w2t = cbt[:, CB_W2:CB_W2 + 128]
        identt = cbt[:, CB_ID:CB_ID + 128]
        vecst = cft
        post = pbt[:, PB_POST:PB_POST + 128]
        prt = pbt[:, PB_POSR:].rearrange("p (k q) -> p k q", k=9)

        # ---- x input: one resident fp16 tile, 7 chunk DMAs (sync queue)
        xfull = xpool.tile([128, 2, NPIX], BF16, tag="xfull")
        xin_r = xind[:].rearrange("(t p) q -> p t q", t=2)
        for c in range(NCH):
            sl = slice(CCH * c, CCH * (c + 1))
            nc.sync.dma_start(xfull[:, :, sl], xin_r[:, :, sl])

        # ---- band-layout tiles + zero borders (memsets on idle GpSimd)
        x1b = sbp.tile([128, Q], BF16, tag="x1b")
        x2b = sbp.tile([128, 16, 58], BF16, tag="x2b")
        x2bB = sbp.tile([128, 16, 58], BF16, tag="x2bB")
        x3sf = sbp.tile([128, 2, 58, 58], BF16, tag="x3sf")
        xgA = sbp.tile([128, SHARE, 16, 58], BF16, tag="xgA")
        nc_.gpsimd.memset(x2b[:], 0.0)
        nc_.gpsimd.memset(x2bB[:], 0.0)
        for t in range(2):
            nc_.gpsimd.memset(x3sf[:, t, 0:1, :], 0.0)      # top border row
            nc_.gpsimd.memset(x3sf[:, t, 57:58, :], 0.0)    # bottom border row
            nc_.gpsimd.memset(x3sf[:, t, 1:57, 0:1], 0.0)   # left border col
            nc_.gpsimd.memset(x3sf[:, t, 1:57, 57:58], 0.0)  # right border col

        # ---- PE warmup on UNINITIALIZED SBUF: zero dependencies, so the
        # matmuls start the moment the PE engine boots (garbage/NaN results
        # land in a dummy PSUM bank that is never read). ~6.4us of activity
        # bridges the input-DMA wait and unthrottles HAM to 2.4 GHz before
        # conv1 starts.
        wpool = ctx.enter_context(tc.tile_pool(name="wpool", bufs=1))
        wsrc = wpool.tile([128, 128], BF16, tag="wsrc")
        nc_.vector.memset(wsrc[:], 0.0)   # DVE is idle at boot
        wexp = wpool.tile([1, 8], F32, tag="wexp")
        nc_.vector.memset(wexp[:], 0.0)
        wdst = wpool.tile([1, 8], F32, tag="wdst")
        nc_.scalar.activation(wdst[:], wexp[:], ACTF.Exp)  # exp table warm
        with tc.tile_pool(name="pwarm", bufs=1, space="PSUM") as pwarm:
            wps_ = pwarm.tile([128, 128], F32, tag="wps")
            for _ in range(60):
                nc_.tensor.matmul(wps_[:], wsrc[:], wsrc[:],
                                  start=True, stop=True)

        # ---- phase A: conv1+conv2 directly in band layout via col-tiling.
        # x1p[32b+j, q]  = conv1 out channel j at band-b pixel q
        # x2p[32b+j, r*56+w] = conv2 out channel j at img (14b-1+r, w)
        pscope0 = ExitStack()
        pp12 = pscope0.enter_context(tc.tile_pool(name="pp12", bufs=1, space="PSUM"))
        x1p = pp12.tile([128, Q], F32, tag="x1p")
        x2p = pp12.tile([128, 16 * W], F32, tag="x2p")
        for b in range(NB):
            bsl = slice(32 * b, 32 * b + 32)
            tp = (0, 32 * b)
            # conv1: band pixels 784b..784(b+1), bank-safe col splits
            for (c0, cw) in ((0, 512), (512, Q - 512)):
                for t in range(2):
                    nc_.tensor.matmul(
                        x1p[bsl, c0:c0 + cw], c1t[:, t, :],
                        xfull[:, t, Q * b + c0:Q * b + c0 + cw],
                        start=(t == 0), stop=(t == 1), tile_position=tp)
            # conv2: rows 14b-1..14b+15 (halo), invalid edge rows skipped
            p0 = (BH * b - 1) * W            # img pixel of dst col 0
            dlo = W if b == 0 else 0         # first valid dst col
            dhi = 16 * W - W if b == NB - 1 else 16 * W
            splits = [(d0, d1) for (d0, d1) in ((dlo, 512), (512, dhi))
                      if d1 > d0]
            for (d0, d1) in splits:
                for t in range(2):
                    nc_.tensor.matmul(
                        x2p[bsl, d0:d1], c2t[:, t, :],
                        xfull[:, t, p0 + d0:p0 + d1],
                        start=(t == 0), stop=(t == 1), tile_position=tp)
        # evictions (ACT, fp16, bias folded): x1b flat; x2b/x2bB as 16x58
        # zero-border planes, data at col offset 1 (x2b) / 2 (x2bB)
        nc_.scalar.activation(x1b[:], x1p[:], ACTF.Identity, bias=vecst[:, 0:1])

        def evict_x2(xt, co_, eng):
            if eng is nc_.scalar:
                eng.activation(xt[:, 1:15, co_:co_ + W],
                               x2p[:, W:15 * W].rearrange(
                                   "p (r w) -> p r w", w=W),
                               ACTF.Identity, bias=vecst[:, 1:2])
            else:
                eng.tensor_scalar(xt[:, 1:15, co_:co_ + W],
                                  x2p[:, W:15 * W].rearrange(
                                      "p (r w) -> p r w", w=W),
                                  vecst[:, 1:2], None, op0=ALU.add)
            # edge rows. row 15 (valid for bands 0..2) is one op at base
            # partition 0; row 0 (bands 1..3) must be split per band (PSUM
            # APs at partition offset p>0 may not span more than 32
            # partitions)
            edges = [(slice(0, 96), 15, slice(15 * W, 16 * W))] + \
                    [(slice(32 * b, 32 * b + 32), 0, slice(0, W))
                     for b in range(1, NB)]
            for (bsl, r, ssl) in edges:
                if eng is nc_.scalar:
                    eng.activation(xt[bsl, r:r + 1, co_:co_ + W],
                                   x2p[bsl, ssl].unsqueeze(1),
                                   ACTF.Identity, bias=vecst[bsl, 1:2])
                else:
                    eng.tensor_scalar(xt[bsl, r:r + 1, co_:co_ + W],
                                      x2p[bsl, ssl].unsqueeze(1),
                                      vecst[bsl, 1:2], None, op0=ALU.add)

        evict_x2(x2b, 1, nc_.scalar)
        evict_x2(x2bB, 2, nc_.scalar)
        pscope0.close()

        # ---- phase B: conv3 (evictions split ACT/DVE; xgA remaps per band)
        pscope1 = ExitStack()
        ppc = pscope1.enter_context(tc.tile_pool(name="ppc", bufs=2, space="PSUM"))
        for c in range(NCH):
            sl = slice(CCH * c, CCH * (c + 1))
            for t in range(2):
                ps3 = ppc.tile([128, CCH], F32, tag="conv")
                nc_.tensor.matmul(ps3[:], c3t[:, 0, 128 * t:128 * (t + 1)],
                                  xfull[:, 0, sl], start=True, stop=False)
                nc_.tensor.matmul(ps3[:], c3t[:, 1, 128 * t:128 * (t + 1)],
                                  xfull[:, 1, sl], start=False, stop=True)
                dst3 = x3sf[:, t, 1 + 8 * c:1 + 8 * c + 8, 1:57]
                src3 = ps3[:].rearrange("p (r w) -> p r w", w=W)
                if t == 0:
                    nc_.scalar.activation(dst3, src3, ACTF.Identity,
                                          bias=vecst[:, 2:3])
                else:
                    nc_.vector.tensor_scalar(dst3, src3, vecst[:, 3:4], None,
                                             op0=ALU.add)
            # xg band remaps: band b needs x3sf plane rows 14b..14b+16
            # xgA[32b+16t+g, s, r, w] = x3sf[8g+s, t, 14b+r, w]
            # (contiguous 928-elem runs per partition -> cheap descriptors)
            for b in range(NB):
                if BAND_LAST_CHUNK[b] == c:
                    for t in range(2):
                        psl = slice(32 * b + 16 * t, 32 * b + 16 * t + 16)
                        nc.sync.dma_start(
                            xgA[psl],
                            x3sf[:, t, 14 * b:14 * b + 16, :])
        pscope1.close()

        xgo = {-1: 0, 1: 2}   # xgA col offset of the dh-window, dw != 0
        x2 = {-1: x2b, 0: x2bB, 1: x2b}
        x2o = {-1: 0, 0: 2, 1: 2}

        # ---- phase C: per-k logits + exp + Z (PE fp16 + ACT)
        frpool = ctx.enter_context(tc.tile_pool(name="frpool", bufs=4))
        hpool = ctx.enter_context(tc.tile_pool(name="hpool", bufs=3))
        eshpool = ctx.enter_context(tc.tile_pool(name="eshpool", bufs=3))
        esh_d = {}
        pscope2 = ExitStack()
        ppz = pscope2.enter_context(tc.tile_pool(name="ppz", bufs=1, space="PSUM"))
        pscope3 = ExitStack()
        pph = pscope3.enter_context(tc.tile_pool(name="pph", bufs=3, space="PSUM"))
        ek = {}
        zps = ppz.tile([128, 1024], F32, tag="zps")
        wsl = [slice(0, 512), slice(512, Q)]
        # software-pipelined: iteration i emits stage-1 work for k_i and
        # stage-2 work for k_{i-1}, so no engine FIFO head-of-line blocks.
        hp_d = {}
        for i in range(10):
            if i < 9:
                k = OFFS_ORD[i]
                dh, dw = OFFS[k]
                co_ = x2o[dw]
                fs = frpool.tile([128, BH, W], BF16, tag="fs")
                nc_.vector.tensor_tensor(
                    fs[:], x1b[:].rearrange("p (r w) -> p r w", w=W),
                    x2[dw][:, 1 + dh:1 + dh + BH, co_:co_ + W],
                    ALU.subtract)
                fr = frpool.tile([128, Q], BF16, tag="fr")
                nc_.vector.tensor_scalar(fr[:].rearrange("p (r w) -> p r w", w=W),
                                         fs[:], 0.0, None, op0=ALU.max)
                hps = pph.tile([128, 1024], F32, tag="hw")
                # same-stationary matmuls back-to-back (one weight load/pair)
                for s in wsl:
                    nc_.tensor.matmul(hps[:, s], w1t[:], fr[:, s],
                                      start=True, stop=False)
                for s in wsl:
                    nc_.tensor.matmul(hps[:, s], post[:], prt[0:8, k, s],
                                      start=False, stop=True)
                hp = hpool.tile([128, Q], BF16, tag="hp")
                hp_d[k] = hp
                # hp eviction fully on DVE: ACT is the phase-C critical
                # chain (it owns all the exp evictions)
                nc_.vector.tensor_scalar(hp[:], hps[:, 0:Q],
                                         vecst[:, 4:5], 0.0,
                                         op0=ALU.add, op1=ALU.max)
            if i >= 1:
                kp = OFFS_ORD[i - 1]
                wps = pph.tile([128, 1024], F32, tag="hw")
                for s in wsl:
                    nc_.tensor.matmul(wps[:, s], w2t[:], hp_d[kp][:, s],
                                      start=True, stop=True)
                e = epool.tile([128, Q], BF16, tag="e")
                nc_.scalar.activation(e[:], wps[:, 0:Q], ACTF.Exp,
                                      bias=vecst[:, 5:6])
                if OFFS[kp][1] == 0:
                    # dw=0 planes also need e at odd column parity: a second
                    # ACT write into a 58-wide row layout at col offset 1
                    # (esh[g, r, 1+w] = e[g, r, w]; cols 0/57 stay garbage,
                    # never read)
                    esh = eshpool.tile([128, BH, 58], BF16, tag="esh")
                    esh_d[kp] = esh
                    # copy from the fp16 e tile (cheaper ACT op than a
                    # second exp eviction from the f32 PSUM)
                    nc_.scalar.activation(
                        esh[:, :, 1:57],
                        e[:].rearrange("p (r w) -> p r w", w=W),
                        ACTF.Identity)
                for s in wsl:
                    nc_.tensor.matmul(zps[:, s], identt[:], e[:, s],
                                      start=(i == 1), stop=(i == 9))
                ek[kp] = e
        pscope3.close()

        # ---- phase E: aggregation + convo, per half-band row group qp.
        # The rz = exp(-ln(Z)) computation (and its ACT table swaps) is
        # emitted AFTER the first quad's multiply/matmul rounds so the two
        # table loads hide under quad-0 compute.
        rzpool = ctx.enter_context(tc.tile_pool(name="rzpool", bufs=1))
        lnz = rzpool.tile([128, Q], F32, tag="lnz")
        rz16 = rzpool.tile([128, Q], BF16, tag="rz16")
        pkpool = ctx.enter_context(tc.tile_pool(name="pkpool", bufs=6))
        sqf = sbp.tile([128, SHARE, Q], BF16, tag="sqf")
        lkpool = ctx.enter_context(tc.tile_pool(name="lkpool", bufs=3))
        opool = ctx.enter_context(tc.tile_pool(name="opool", bufs=4))
        samsb = sbp.tile([128, 2, NPIX], BF16, tag="samsb")
        pscope4 = ExitStack()
        pps = pscope4.enter_context(tc.tile_pool(name="pps", bufs=1, space="PSUM"))
        ppo = pscope4.enter_context(tc.tile_pool(name="ppo", bufs=2, space="PSUM"))

        first_quad = [True]

        def emit_pk(k, qp, sq, engine):
            # pk plane for offset k, quad (qp, sq); returns matmul rhs list
            dh, dw = OFFS[k]
            r0 = 1 + dh + 7 * qp
            sqs = slice(NSQ * sq, NSQ * (sq + 1))
            if dw != 0:
                pk = pkpool.tile([128, NSQ, 7, W], BF16, tag="pk")
                co_ = xgo[dw]
                engine.tensor_tensor(
                    pk[:],
                    xgA[:, sqs, r0:r0 + 7, co_:co_ + W],
                    ek[k][:].rearrange("p (r w) -> p r w", w=W)
                    [:, 7 * qp:7 * qp + 7, :].unsqueeze(1)
                    .broadcast_to((128, NSQ, 7, W)),
                    ALU.mult)
                return [pk[:, c4, :, :] for c4 in range(4)]
            # dw=0: full 58-wide product against the shifted-e copy (both
            # operands at even parity); the identity matmul reads the
            # odd-offset window instead
            pk = pkpool.tile([128, NSQ, 7, 58], BF16, tag="pk8")
            engine.tensor_tensor(
                pk[:],
                xgA[:, sqs, r0:r0 + 7, 0:58],
                esh_d[k][:, 7 * qp:7 * qp + 7, :].unsqueeze(1)
                .broadcast_to((128, NSQ, 7, 58)),
                ALU.mult)
            return [pk[:, c4, :, 1:57] for c4 in range(4)]

        for qp in range(2):
            for sq in range(2):
                sam = pps.tile([128, 2048], F32, tag="sam")
                samv = sam[:].rearrange("p (a j) -> p a j", j=512)[:, :, 0:HBW]
                for j, k in enumerate(OFFS_ORD):
                    rhs = emit_pk(k, qp, sq, nc_.vector)
                    for c4 in range(4):
                        nc_.tensor.matmul(
                            sam[:, 512 * c4:512 * c4 + HBW], identt[:],
                            rhs[c4],
                            start=(j == 0), stop=(j == 8))
                if first_quad[0]:
                    # rz = exp(-ln(Z)): table loads overlap quad-0 compute
                    nc_.scalar.activation(lnz[:], zps[:, 0:Q], ACTF.Ln)
                    nc_.scalar.activation(rz16[:], lnz[:], ACTF.Exp,
                                          scale=-1.0)
                    first_quad[0] = False
                # leaky(sam) then * rz -> sqf quad slice
                lk = lkpool.tile([128, NSQ, HBW], BF16, tag="lk")
                nc_.scalar.activation(lk[:], samv, ACTF.Prelu, alpha=NEG)
                nc_.vector.tensor_tensor(
                    sqf[:, NSQ * sq:NSQ * (sq + 1), HBW * qp:HBW * (qp + 1)],
                    lk[:],
                    rz16[:, HBW * qp:HBW * (qp + 1)].unsqueeze(1)
                    .broadcast_to((128, NSQ, HBW)),
                    ALU.mult)
                # sam remap for this s-quad: dst partitions 64sq..64sq+64
                # are contiguous (convo lhsT rows permuted to match), so the
                # transfer starts one quad earlier than a full-s remap.
                # dst [64, 392] pairs with src [16, 4, 392] in flat order:
                # dst 64sq+4g+smod <- sqf[32b+16t+g, 4sq+smod]
                for t in range(2):
                    for b in range(NB):
                        # the last quad's issues split across both HWDGE
                        # queues: ACT has no independent work left there,
                        # so head-of-line blocking is harmless and the
                        # 8x ~0.7us issue serialization halves
                        q_ = (nc.scalar if qp == 1 and sq == 1
                              and (t + b) % 2 == 1 else nc.sync)
                        q_.dma_start(
                            samsb[64 * sq:64 * sq + 64, t,
                                  Q * b + HBW * qp:Q * b + HBW * (qp + 1)],
                            sqf[32 * b + 16 * t:32 * b + 16 * t + 16,
                                NSQ * sq:NSQ * (sq + 1),
                                HBW * qp:HBW * (qp + 1)])
            # convo on this qp's half-bands; one merged store per band
            # (residual +x is added on the host in f32)
            for b in range(NB):
                po = Q * b + HBW * qp
                o = opool.tile([128, 2, HBW], BF16, tag="o")
                for to in range(2):
                    pso = ppo.tile([128, HBW], F32, tag="pso")
                    nc_.tensor.matmul(pso[:], cot[:, 0, 128 * to:128 * (to + 1)],
                                      samsb[:, 0, po:po + HBW],
                                      start=True, stop=False)
                    nc_.tensor.matmul(pso[:], cot[:, 1, 128 * to:128 * (to + 1)],
                                      samsb[:, 1, po:po + HBW],
                                      start=False, stop=True)
                    nc_.scalar.activation(o[:, to, :], pso[:], ACTF.Prelu,
                                          bias=vecst[:, 6 + to:7 + to],
                                          alpha=NEG)
                nc.sync.dma_start(
                    outd[:].rearrange("(t p) q -> p t q", t=2)
                    [:, :, po:po + HBW], o[:])
        pscope4.close()
        pscope2.close()

        if DEBUG:
            nc.sync.dma_start(dbg["d_x1b"][:], x1b[:])
            nc.sync.dma_start(dbg["d_x2b"][:],
                              x2b[:].rearrange("p r w -> p (r w)"))
            nc.sync.dma_start(dbg["d_x2bB"][:],
                              x2bB[:].rearrange("p r w -> p (r w)"))
            nc.sync.dma_start(dbg["d_x3sf"][:],
                              x3sf[:].rearrange("p t r w -> p (t r w)"))
            nc.sync.dma_start(dbg["d_e0"][:], ek[0][:])
            nc.sync.dma_start(dbg["d_e4"][:], ek[4][:])
            nc.sync.dma_start(dbg["d_rz"][:], rz16[:])
            nc.sync.dma_start(dbg["d_sqf"][:],
                              sqf[:].rearrange("p s q -> p (s q)"))
            nc.sync.dma_start(dbg["d_samsb"][:],
                              samsb[:].rearrange("p t q -> p (t q)"))
            nc.sync.dma_start(dbg["d_xga"][:],
                              xgA[:].rearrange("p s r w -> p (s r w)"))

    nc.compile()
    return nc


# --------------------------------------------------------------- entrypoint
def _get_program():
    if "nc" not in _CACHE:
        _CACHE["nc"] = _build_program()
    return _CACHE["nc"]


def _run(inputs, trace):
    inputs = {k: np.asarray(v) for k, v in inputs.items()}
    consts = _host_consts(inputs)
    nc = _get_program()
    x = inputs["x"].reshape(B, CIN, NPIX).astype(np.float16)
    in_maps = []
    for b in range(B):
        m = {k: v for k, v in consts.items()}
        m["xin"] = x[b]
        in_maps.append(m)
    res = run_bass_kernel_spmd(nc, in_maps, list(range(B)), trace=trace)
    out = np.stack([res.results[i]["out"] for i in range(B)])
    out = out.reshape(B, CIN, H, W).astype(np.float32)
    out += inputs["x"].astype(np.float32)  # residual in f32 on host
    return out, res


def kernel(**inputs):
    return _run(inputs, False)[0]


def kernel_traced(**inputs):
    """Like kernel() but with NTFF tracing; returns (out, BassKernelResults)."""
    return _run(inputs, True)
